# revision 15
# baseline (speedup 1.0000x reference)
"""Causal self-attention with RoPE on 8 Trainium2 NeuronCores.

Sharding: Megatron-style head parallelism. 16 heads / 8 cores = 2 heads per
core. Each core computes q/k/v projections for its 2 heads (column-parallel),
full causal attention for those heads, and a partial output projection
(row-parallel slice of w_o). The host sums the 8 partial outputs.

On-chip layout: everything transposed. Host passes xT = x^T per batch
[B, D, T]; projections produce qT/kT [dh, t] directly (lhsT = weight slice,
rhs = xT chunk) and v [t, dh] (lhsT = xT chunk, rhs = w_v slice). Scores are
computed transposed, ST[kv, q] = matmul(lhsT=kT_chunk, rhs=qT_group), which
makes P^T directly usable as the moving operand of the PV matmul - no
on-chip transposes anywhere.

All matmul operands are float32r (full PE rate at moving-dim >= 256, and -
unlike bf16 - the fp32r matmul self-loads its stationary, overlapping the
weight load with the stream; bf16 operands emit separate LDWEIGHTS
instructions that serialize ~130ns per dependent matmul, measured).

Structural changes over the 952us baseline:
- PV + denominator-sum matmuls are emitted one kv-tile BEHIND the score
  matmul, so the ACT exp latency (~750ns) hides under the next score
  instead of stalling the PE (was 54us of in-slice PE waits).
- Causal mask: binary 0/1 multiply on the exp'd tile (DVE, in-place in
  SBUF) instead of an identity-matmul mask-add on the PE (-30us PE).
- Softmax reciprocal as exp(-ln(den)) on ACT [1,TQ] (Ln/Exp/Copy share one
  activation table set, so no table reloads); the baseline's
  nc.vector.reciprocal on [128,TQ] cost 3.3us x32 = 107us of DVE.
- The normalize chain is emitted inline at q-group end (its ACT ops queue
  ahead of the next group's exps), and only the out-projection is deferred
  one q-group; the last groups of a batch drain inside the NEXT batch's
  projection phase instead of stalling the PE at batch transitions.
- TT=512 projections (half the matmul instruction count of TT=256); x
  tiles are split into two half-depth chunks to fit SBUF.
- wo loads are deferred until after batch 0's x tiles in the DMA queues.
- Output y is written in bf16 (halves the output DMA); the host upcasts
  and sums the 8 partials.

The attention scale 1/sqrt(dh) is folded into w_q on the host; the RoPE
rotate-half is two partition-offset multiplies with a sign-folded sin
table (PSUM-sourced: the DVE may cross partitions only on PSUM operands).
No max-subtraction: logits are q.k/sqrt(dh) with unit-ish variance,
|logit| << 88, identical math to the reference.
"""

import numpy as np

B, T, D = 4, 2048, 2048
H, DH = 16, 128
NCORES = 8
HPC = H // NCORES  # heads per core
THETA = 10000.0

TT = 512  # projection t-tile (moving dim of q/k projection matmuls)
TQ = 512  # attention q-group width
TK = 128  # kv tile (contraction chunk of PV / partition dim of ST)


def _rope_tables(seq_len, d_head, theta):
    # Matches reference.rope_cos_sin numerics, then transposes to [dh, t]
    # and folds the rotate-half sign into sin.
    inv_freq = 1.0 / (theta ** (np.arange(0, d_head, 2, dtype=np.float32) / d_head))
    t = np.arange(seq_len, dtype=np.float32)
    freqs = np.einsum("i,j->ij", t, inv_freq)
    emb = np.concatenate([freqs, freqs], axis=-1)  # [T, dh]
    cosT = np.ascontiguousarray(np.cos(emb).astype(np.float32).T)  # [dh, T]
    sinT = np.ascontiguousarray(np.sin(emb).astype(np.float32).T)
    sgn = np.ones((d_head, 1), np.float32)
    sgn[: d_head // 2] = -1.0
    return cosT, sinT * sgn


def _causal_mask_bin(tq, tk):
    # Binary keep-mask master [tk, (tq-tk)+tq]: slice
    # [:, (tq-tk)-dg*tk :][:tq] covers the diagonal block at offset dg.
    # Entry is 0.0 (masked) iff column j < (tq - tk) + r, else 1.0.
    width = (tq - tk) + tq
    m = np.ones((tk, width), np.float32)
    for r in range(tk):
        m[r, : (tq - tk) + r] = 0.0
    return m


def _legalize_waits(nc, mybir):
    """Walrus on this toolchain refuses more than one embedded sync wait
    per engine instruction. Hoist extra waits into standalone
    EventSemaphore instructions on the same engine queue (the sequencer
    executes them in-stream before the instruction, same gating)."""
    n = 0
    for f in nc.m.functions:
        for bb in f.blocks:
            out = []
            for inst in bb.instructions:
                si = inst.sync_info
                if (si and si.on_wait and len(si.on_wait) > 1
                        and not isinstance(inst, mybir.InstEventSemaphore)):
                    for w in si.on_wait[:-1]:
                        out.append(mybir.InstEventSemaphore(
                            name=f"WH-{n}", engine=inst.engine,
                            sync_info=mybir.SyncInfo(
                                on_wait=[w], on_update=[])))
                        n += 1
                    inst.sync_info = mybir.SyncInfo(
                        on_wait=[si.on_wait[-1]],
                        on_update=list(si.on_update))
                out.append(inst)
            bb.instructions = out
    return n


def _build_nc(b_sz, t_sz, d_sz, legalize=True):
    import concourse.bass as bass
    import concourse.tile as tile
    from concourse import mybir

    f32 = mybir.dt.float32
    f32r = mybir.dt.float32r
    bf16 = mybir.dt.bfloat16
    EXP = mybir.ActivationFunctionType.Exp
    LN = mybir.ActivationFunctionType.Ln

    DC = d_sz // 128         # contraction chunks
    DCH = DC // 2            # chunks per x half-tile
    NQG = t_sz // TQ         # q groups per (batch, head)
    NKT = t_sz // TK         # kv tiles
    KPG = TQ // TK           # kv tiles per q group (diagonal span)
    MW = (TQ - TK) + TQ      # mask master width

    nc = bass.Bass("TRN2", target_bir_lowering=False, debug=False,
                   enable_asserts=False, dynamic_dma_scratch_size=2048)

    xT = nc.dram_tensor("xT", [b_sz, d_sz, t_sz], f32, kind="ExternalInput")
    wq = nc.dram_tensor("wq", [d_sz, HPC * DH], f32, kind="ExternalInput")
    wk = nc.dram_tensor("wk", [d_sz, HPC * DH], f32, kind="ExternalInput")
    wv = nc.dram_tensor("wv", [d_sz, HPC * DH], f32, kind="ExternalInput")
    wo = nc.dram_tensor("wo", [HPC * DH, d_sz], f32, kind="ExternalInput")
    cos = nc.dram_tensor("cos", [DH, t_sz], bf16, kind="ExternalInput")
    sin = nc.dram_tensor("sin", [DH, t_sz], bf16, kind="ExternalInput")
    msk = nc.dram_tensor("msk", [TK, MW], bf16, kind="ExternalInput")
    one = nc.dram_tensor("one", [128, 128], f32, kind="ExternalInput")
    y = nc.dram_tensor("y", [b_sz, t_sz, d_sz], bf16, kind="ExternalOutput")

    xT_r = xT.ap().rearrange("b (dc p) t -> b p dc t", p=128)
    wq_r = wq.ap().rearrange("(dc p) n -> p dc n", p=128)
    wk_r = wk.ap().rearrange("(dc p) n -> p dc n", p=128)
    wv_r = wv.ap().rearrange("(dc p) n -> p dc n", p=128)
    wo_r = wo.ap().rearrange("(h p) n -> p h n", p=128)
    y_r = y.ap()

    with tile.TileContext(nc) as tc:
        with (
            tc.tile_pool(name="consts", bufs=1) as consts,
            tc.tile_pool(name="wpool", bufs=1) as wpool,
            tc.tile_pool(name="qkv", bufs=1) as qkv,
            tc.tile_pool(name="xpool", bufs=3) as xpool,
            tc.tile_pool(name="rope", bufs=2) as rope,
            tc.tile_pool(name="pex", bufs=3) as pexp,
            tc.tile_pool(name="nrm", bufs=2) as nrmp,
            tc.tile_pool(name="otn", bufs=6) as otnp,
            tc.tile_pool(name="ysb", bufs=6) as ysbp,
            tc.tile_pool(name="psS", bufs=2, space="PSUM") as psS,
            tc.tile_pool(name="psO", bufs=2, space="PSUM") as psO,
            tc.tile_pool(name="psR", bufs=1, space="PSUM") as psR,
            tc.tile_pool(name="psY", bufs=2, space="PSUM") as psY,
        ):
            cos_sb = consts.tile([DH, t_sz], bf16)
            sin_sb = consts.tile([DH, t_sz], bf16)
            msk_sb = consts.tile([TK, MW], bf16)
            ones_sb = consts.tile([128, 1], f32r)
            onesrow_sb = consts.tile([1, 128], f32r)

            wq_sb = wpool.tile([128, DC, HPC * DH], f32r)
            wk_sb = wpool.tile([128, DC, HPC * DH], f32r)
            wv_sb = wpool.tile([128, DC, HPC * DH], f32r)
            wo_sb = wpool.tile([128, HPC, d_sz], f32r)

            def load_x_half(xh, b, half, tsl):
                for dc in range(DCH):
                    nc.sync.dma_start(
                        xh[:, dc, :],
                        xT_r[b, :, half * DCH + dc, tsl].bitcast(f32r))

            # first-needed data first: the first x half-tile and q weight
            # chunks feed the very first matmuls, so their DMAs go at the
            # head of every queue; wk/wv/cos/sin follow in consumption
            # order.
            xt_first = [xpool.tile([128, DCH, TT], f32r, tag="xt",
                                   name="xt_first") for _ in range(2)]
            for dc in range(DCH):
                nc.sync.dma_start(xt_first[0][:, dc, :],
                                  xT_r[0, :, dc, 0:TT].bitcast(f32r))
                nc.sync.dma_start(wq_sb[:, dc, :],
                                  wq_r[:, dc, :].bitcast(f32r))
            for dc in range(DCH):
                nc.sync.dma_start(xt_first[1][:, dc, :],
                                  xT_r[0, :, DCH + dc, 0:TT].bitcast(f32r))
                nc.sync.dma_start(wq_sb[:, DCH + dc, :],
                                  wq_r[:, DCH + dc, :].bitcast(f32r))
            nc.sync.dma_start(cos_sb[:, 0:TT], cos.ap()[:, 0:TT])
            nc.sync.dma_start(sin_sb[:, 0:TT], sin.ap()[:, 0:TT])
            for dc in range(DC):
                nc.sync.dma_start(wk_sb[:, dc, :],
                                  wk_r[:, dc, :].bitcast(f32r))
            for dc in range(DC):
                nc.sync.dma_start(wv_sb[:, dc, :],
                                  wv_r[:, dc, :].bitcast(f32r))

            def load_consts():
                # emitted after the first x tile's DMAs: nothing here is
                # needed before attention of the first tile
                for i in range(1, t_sz // TT):
                    sl = slice(i * TT, (i + 1) * TT)
                    nc.sync.dma_start(cos_sb[:, sl], cos.ap()[:, sl])
                    nc.sync.dma_start(sin_sb[:, sl], sin.ap()[:, sl])
                nc.sync.dma_start(msk_sb[:], msk.ap())
                nc.sync.dma_start(ones_sb[:], one.ap()[:, 0:1].bitcast(f32r))
                nc.sync.dma_start(onesrow_sb[:],
                                  one.ap()[0:1, :].bitcast(f32r))

            def load_wo():
                # deferred past all of batch 0's x tiles so the 8.4MB of wo
                # doesn't sit ahead of them in the DMA queue FIFOs; first
                # needed by the first out-projection, ~25us into phase B
                for hh in range(HPC):
                    for nch in range(d_sz // 512):
                        nsl = slice(nch * 512, (nch + 1) * 512)
                        nc.sync.dma_start(wo_sb[:, hh, nsl],
                                          wo_r[:, hh, nsl].bitcast(f32r))

            pending = []
            norm_pending = []
            otn_tiles = {}
            xt_prefetch = {}

            for b in range(b_sz):
                # ---------------- phase A: projections + RoPE ----------
                qT = [qkv.tile([DH, t_sz], f32r, tag=f"qT{h}", name=f"qT{h}")
                      for h in range(HPC)]
                kT = [qkv.tile([DH, t_sz], f32r, tag=f"kT{h}", name=f"kT{h}")
                      for h in range(HPC)]
                vv = qkv.tile([128, NKT, HPC * DH], f32r, tag="vv",
                              name="vv")

                for tt in range(t_sz // TT):
                    tsl = slice(tt * TT, (tt + 1) * TT)
                    if b == 0 and tt == 0:
                        xt = xt_first
                        load_consts()
                    elif (b, tt) in xt_prefetch:
                        xt = xt_prefetch.pop((b, tt))
                    else:
                        xt = [xpool.tile([128, DCH, TT], f32r, tag="xt",
                                         name="xt") for _ in range(2)]
                        load_x_half(xt[0], b, 0, tsl)
                        load_x_half(xt[1], b, 1, tsl)

                    for h in range(HPC):
                        hs = slice(h * DH, (h + 1) * DH)
                        for dst, w_sb in ((qT[h], wq_sb), (kT[h], wk_sb)):
                            pp = psS.tile([128, TT], f32, tag="st", name="pp")
                            for dc in range(DC):
                                nc.tensor.matmul(
                                    pp[:],
                                    w_sb[:, dc, hs],
                                    xt[dc // DCH][:, dc % DCH, :],
                                    start=(dc == 0), stop=(dc == DC - 1),
                                )
                            # RoPE: dst = ppc*cos + swap(ppc)*sin_signed.
                            # The pp PSUM bank is freed by a fast ACT copy
                            # (if the DVE reads pp directly, the st-ring
                            # couples the PE to DVE backlog: measured
                            # ~3.9us PE stalls per occurrence plus p-state
                            # resets). The rotate-half partition swap runs
                            # on the DMA engine (the DVE cannot pair SBUF
                            # operands at different start partitions).
                            ppc = rope.tile([128, TT], bf16, tag="ppc",
                                            name="ppc")
                            nc.scalar.copy(ppc[:], pp[:])
                            psw = rope.tile([128, TT], bf16, tag="psw",
                                            name="psw")
                            nc.sync.dma_start(psw[0:64, :], ppc[64:128, :])
                            nc.sync.dma_start(psw[64:128, :], ppc[0:64, :])
                            sh = rope.tile([128, TT], bf16, tag="sh",
                                           name="sh")
                            nc.vector.tensor_mul(sh[:], psw[:],
                                                 sin_sb[:, tsl])
                            nc.vector.tensor_mul(dst[:, tsl], ppc[:],
                                                 cos_sb[:, tsl])
                            nc.vector.tensor_add(dst[:, tsl], dst[:, tsl],
                                                 sh[:])

                    for ts2 in range(TT // TK):
                        vp = psS.tile([TK, HPC * DH], f32, tag="st",
                                      name="vp")
                        for dc in range(DC):
                            nc.tensor.matmul(
                                vp[:],
                                xt[dc // DCH][:, dc % DCH,
                                              ts2 * TK:(ts2 + 1) * TK],
                                wv_sb[:, dc, :],
                                start=(dc == 0), stop=(dc == DC - 1),
                            )
                        kv_i = tt * (TT // TK) + ts2
                        nc.scalar.copy(vv[:, kv_i, :], vp[:])

                    # drain deferred norm chains / out-projections from
                    # the previous batch under this batch's projections
                    if norm_pending:
                        norm_pending.pop(0)()
                    if pending:
                        pending.pop(0)()

                if b == 0:
                    load_wo()

                # prefetch the next batch's first x tile: emitted here so
                # its DMAs sit ahead of this batch's y-output traffic in
                # the queues (the ring slots it takes were freed by this
                # batch's phase A, so it never head-of-line blocks)
                if b + 1 < b_sz:
                    xt_nb = [xpool.tile([128, DCH, TT], f32r, tag="xt",
                                        name="xt") for _ in range(2)]
                    load_x_half(xt_nb[0], b + 1, 0, slice(0, TT))
                    load_x_half(xt_nb[1], b + 1, 1, slice(0, TT))
                    xt_prefetch[(b + 1, 0)] = xt_nb

                # ---------------- phase B + C: attention + out proj ----
                for h in range(HPC):
                    for qi in range(NQG):
                        nkv = KPG * (qi + 1)
                        outp = psO.tile([DH, TQ], f32, tag="outT",
                                        name="outp")
                        sump = psR.tile([1, TQ], f32, tag="sums",
                                        name="sump")
                        prev_pex = None
                        for ki in range(nkv):
                            stp = psS.tile([TK, TQ], f32, tag="st",
                                           name="stp")
                            nc.tensor.matmul(
                                stp[:],
                                kT[h][:, ki * TK:(ki + 1) * TK],
                                qT[h][:, qi * TQ:(qi + 1) * TQ],
                                start=True, stop=True,
                            )
                            # PV+sum of the PREVIOUS kv tile go to the PE
                            # now, so exp(ki) hides under score(ki+1)
                            if prev_pex is not None:
                                pki, ppex = prev_pex
                                nc.tensor.matmul(
                                    outp[:], vv[:, pki, h * DH:(h + 1) * DH],
                                    ppex[:], start=(pki == 0), stop=False,
                                )
                                nc.tensor.matmul(
                                    sump[:], ones_sb[:],
                                    ppex[:], start=(pki == 0), stop=False,
                                )
                            pex = pexp.tile([TK, TQ], f32r, tag="pex",
                                            name="pex")
                            nc.scalar.activation(pex[:], stp[:], EXP)
                            dg = ki - KPG * qi
                            if dg >= 0:
                                off = (TQ - TK) - dg * TK
                                nc.gpsimd.tensor_mul(
                                    pex[:], pex[:], msk_sb[:, off:off + TQ])
                            prev_pex = (ki, pex)
                            if ki == 0 and norm_pending:
                                norm_pending.pop(0)()
                        pki, ppex = prev_pex
                        nc.tensor.matmul(
                            outp[:], vv[:, pki, h * DH:(h + 1) * DH],
                            ppex[:], start=(pki == 0), stop=True,
                        )
                        nc.tensor.matmul(
                            sump[:], ones_sb[:],
                            ppex[:], start=(pki == 0), stop=True,
                        )

                        # normalization chain, deferred one kv-tile into
                        # the NEXT q-group so its ACT ops (LN, exp(-x))
                        # never sit between consecutive exps at a group
                        # boundary (engines run their queues in order, so
                        # anything inserted there delays the exp the PE is
                        # about to need). rcp = exp(-ln(den)) runs on ACT
                        # (same table set as Exp/Copy: no ACT_TABLE_LOAD);
                        # a ones-matmul broadcasts it across partitions.
                        def norm_group(h=h, qi=qi, outp=outp, sump=sump):
                            lnd = nrmp.tile([1, TQ], f32, tag="lnd",
                                            name="lnd")
                            nc.scalar.activation(lnd[:], sump[:], LN)
                            rcp1 = nrmp.tile([1, TQ], f32r, tag="rcp1",
                                             name="rcp1")
                            nc.scalar.activation(rcp1[:], lnd[:], EXP,
                                                 scale=-1.0)
                            rbc = psR.tile([DH, TQ], f32, tag="rbc",
                                           name="rbc")
                            nc.tensor.matmul(rbc[:], onesrow_sb[:], rcp1[:],
                                             start=True, stop=True)
                            rbs = nrmp.tile([DH, TQ], bf16, tag="rbs",
                                            name="rbs")
                            nc.vector.tensor_copy(rbs[:], rbc[:])
                            otn = otnp.tile([DH, TQ], f32r, tag="otn",
                                            name="otn")
                            nc.vector.tensor_mul(otn[:], outp[:], rbs[:])
                            otn_tiles[(h, qi)] = otn

                        norm_pending.append(norm_group)

                        if h == HPC - 1:
                            def out_project(qi=qi, b=b):
                                for tc2 in range(TQ // TK):
                                    tq0 = qi * TQ + tc2 * TK
                                    for nch in range(d_sz // 512):
                                        yp = psY.tile([TK, 512], f32,
                                                      tag="y", name="yp")
                                        for hh in range(HPC):
                                            nc.tensor.matmul(
                                                yp[:],
                                                otn_tiles[(hh, qi)][
                                                    :, tc2 * TK:(tc2 + 1) * TK],
                                                wo_sb[:, hh,
                                                      nch * 512:(nch + 1) * 512],
                                                start=(hh == 0),
                                                stop=(hh == HPC - 1),
                                            )
                                        ysb = ysbp.tile([TK, 512], bf16,
                                                        tag="ysb", name="ysb")
                                        nc.vector.tensor_copy(ysb[:], yp[:])
                                        nc.sync.dma_start(
                                            y_r[b, tq0:tq0 + TK,
                                                nch * 512:(nch + 1) * 512],
                                            ysb[:])

                            pending.append(out_project)
                        if len(pending) > 1:
                            pending.pop(0)()
            for fn in norm_pending:
                fn()
            for fn in pending:
                fn()
    if legalize:
        _legalize_waits(nc, mybir)
    return nc


_NC_CACHE = {}
LAST_RESULT = None


def _get_nc(b_sz, t_sz, d_sz):
    key = (b_sz, t_sz, d_sz)
    if key not in _NC_CACHE:
        _NC_CACHE[key] = _build_nc(b_sz, t_sz, d_sz)
    return _NC_CACHE[key]


def kernel(x, w_q, w_k, w_v, w_o):
    import ml_dtypes
    from concourse.bass_utils import run_bass_kernel_spmd

    BF = ml_dtypes.bfloat16
    b_sz, t_sz, d_sz = x.shape
    scale = np.float32(1.0 / np.sqrt(DH))

    xT = np.ascontiguousarray(np.asarray(x, np.float32).transpose(0, 2, 1))
    w_q = np.asarray(w_q, np.float32)
    w_k = np.asarray(w_k, np.float32)
    w_v = np.asarray(w_v, np.float32)
    w_o = np.asarray(w_o, np.float32)
    cosT, sinT = _rope_tables(t_sz, DH, THETA)
    mask = _causal_mask_bin(TQ, TK)

    in_maps = []
    for c in range(NCORES):
        cs = slice(c * HPC * DH, (c + 1) * HPC * DH)
        in_maps.append({
            "xT": xT,
            "wq": np.ascontiguousarray(w_q[:, cs] * scale),
            "wk": np.ascontiguousarray(w_k[:, cs]),
            "wv": np.ascontiguousarray(w_v[:, cs]),
            "wo": np.ascontiguousarray(w_o[cs, :]),
            "cos": cosT.astype(BF),
            "sin": sinT.astype(BF),
            "msk": mask.astype(BF),
            "one": np.ones((128, 128), np.float32),
        })

    nc = _get_nc(b_sz, t_sz, d_sz)
    res = run_bass_kernel_spmd(nc, in_maps, core_ids=list(range(NCORES)))
    global LAST_RESULT
    LAST_RESULT = res

    out = np.asarray(res.results[0]["y"]).astype(np.float32)
    for c in range(1, NCORES):
        out += np.asarray(res.results[c]["y"]).astype(np.float32)
    return out
